# revision 20
# baseline (speedup 1.0000x reference)
"""Trainium2 Bass kernel for nn_EntityAttentionLayer (sparse attention).

Strategy (8 cores, data-parallel over bs):
  - Host side: shard bs across 8 cores (64 items each), pre-transpose
    entities to E^T[in_dim, ne] per batch, cast operands to bf16,
    convert masks to multiplicative keep-masks.
  - On chip, per batch b (processed in pairs, Q in octets of 8):
      K^T[ed, ne]  = (Wk^T E^T)        via lhsT=Wk slices, rhs=E^T
      V[ne, ed]    = E V-proj          via lhsT=E^T slices, rhs=Wv
      Q^T[ed, q]   =                   via lhsT=Wq slices, rhs=E^T[:, :64]
      logits^T[ne, q] per head         lhsT=K^T_h, rhs=Q^T_h  (ne on partitions)
      wm = exp(logits * 1/sqrt(hd))    on ACT (scale folded into activation)
      wm *= keep^T                     on DVE (multiplicative mask, h-broadcast)
      sums broadcast [128, h*q]        PE matmul with all-ones lhsT
      attn^T unnorm [2-heads, hp*b*q]  lhsT=V slices, rhs=wm  (col-tiled pairs)
      attn = attn_unnorm * 1/sums      DVE (approx reciprocal + strided muls)
      out[b*q, out] = attn^T.T @ W_out (+bias & post-mask fused on DVE)
      out *= keep_post (per-partition scalar), DMA out.
  All matmuls bf16 operands, fp32 PSUM accumulation.
"""

import numpy as np
import ml_dtypes

BS, NE, NQ, IN_DIM, ED, OUT_DIM, H, HD = 512, 256, 64, 512, 512, 512, 8, 64
NCORES = 8
BPC = BS // NCORES          # 64 batches per core
OCT = 8                     # batches per super-batch (Q^T amortization)
NOCT = BPC // OCT           # 8
PAIRS_PER_OCT = OCT // 2    # 4
NPAIRS = BPC // 2           # 32
SCALE = 1.0 / float(np.sqrt(HD))

BF16 = ml_dtypes.bfloat16

_BUILT = {}
LAST_RESULT = None


def _build_nc():
    import concourse.tile as tile
    from concourse import bacc, mybir
    from contextlib import ExitStack

    f32 = mybir.dt.float32
    bf16 = mybir.dt.bfloat16

    nc = bacc.Bacc("TRN2", target_bir_lowering=False)

    eT_d = nc.dram_tensor("eT", [NOCT, 128, OCT, 4, NE], bf16, kind="ExternalInput")
    keep_d = nc.dram_tensor("keep", [NOCT, 128, OCT, 2, NQ], bf16, kind="ExternalInput")
    postm_d = nc.dram_tensor("postm", [128, NPAIRS], f32, kind="ExternalInput")
    ident_d = nc.dram_tensor("ident", [128, 128], bf16, kind="ExternalInput")
    w_in_d = nc.dram_tensor("w_in", [4, 128, 3 * ED], bf16, kind="ExternalInput")
    w_out_d = nc.dram_tensor("w_out", [4, 128, OUT_DIM], bf16, kind="ExternalInput")
    b_out_d = nc.dram_tensor("b_out", [1, OUT_DIM], f32, kind="ExternalInput")
    out_d = nc.dram_tensor("out", [BPC, NQ, OUT_DIM], f32, kind="ExternalOutput")

    with ExitStack() as ctx:
        tc = ctx.enter_context(tile.TileContext(nc))
        consts = ctx.enter_context(tc.tile_pool(name="consts", bufs=1))
        p_eT = ctx.enter_context(tc.tile_pool(name="p_eT", bufs=2))
        p_keep = ctx.enter_context(tc.tile_pool(name="p_keep", bufs=2))
        p_kT = ctx.enter_context(tc.tile_pool(name="p_kT", bufs=3))
        p_v = ctx.enter_context(tc.tile_pool(name="p_v", bufs=3))
        p_wm = ctx.enter_context(tc.tile_pool(name="p_wm", bufs=6))
        p_recip = ctx.enter_context(tc.tile_pool(name="p_recip", bufs=2))
        p_attn = ctx.enter_context(tc.tile_pool(name="p_attn", bufs=2))
        p_out = ctx.enter_context(tc.tile_pool(name="p_out", bufs=3))
        pp = ctx.enter_context(tc.tile_pool(name="pp", bufs=2, space="PSUM"))

        # Constants
        w_sb = consts.tile([128, 4, 3 * ED], bf16)
        for wc in range(3):
            nc.sync.dma_start(
                out=w_sb[:, :, wc * ED:(wc + 1) * ED],
                in_=w_in_d[:, :, wc * ED:(wc + 1) * ED].rearrange("k p n -> p k n"))
        wo_sb = consts.tile([128, 4, OUT_DIM], bf16)
        nc.sync.dma_start(out=wo_sb, in_=w_out_d[:, :, :].rearrange("k p n -> p k n"))
        bias_bc = consts.tile([128, OUT_DIM], f32)
        nc.sync.dma_start(out=bias_bc, in_=b_out_d[:, :].to_broadcast([128, OUT_DIM]))
        postm_sb = consts.tile([128, NPAIRS], f32)
        nc.sync.dma_start(out=postm_sb, in_=postm_d[:, :])
        ones_sb = consts.tile([128, 128], bf16)
        nc.vector.memset(ones_sb, 1.0)
        ident_sb = consts.tile([128, 128], bf16)
        nc.sync.dma_start(out=ident_sb, in_=ident_d[:, :])
        # Persistent zero-padded Q^T tiles (manual double buffer by octet
        # parity). Layout [128, m, h2, b, q]: head parity h2 selects which
        # 64-row half holds data; the other half stays zero so logits
        # matmuls can use full K=128 operands at base partition 0
        # (operands at base partition 64 fault on HW).
        qz0 = consts.tile([128, 4, 2, OCT, HD], bf16)
        nc.vector.memset(qz0, 0.0)
        qz1 = consts.tile([128, 4, 2, OCT, HD], bf16)
        nc.vector.memset(qz1, 0.0)
        qz_bufs = [qz0, qz1]

        for oc in range(NOCT):
            eT_sb = p_eT.tile([128, OCT, 4, NE], bf16, tag="eT")
            for hc in range(2):
                nc.sync.dma_start(
                    out=eT_sb[:, hc * 4:(hc + 1) * 4, :, :],
                    in_=eT_d[oc, :, hc * 4:(hc + 1) * 4, :, :],
                )
            keep_sb = p_keep.tile([128, OCT, 2, NQ], bf16, tag="keep")
            nc.sync.dma_start(out=keep_sb, in_=keep_d[oc, :, :, :, :])

            # ---- Q^T for the whole octet: amortize W_q weight loads ----
            qz = qz_bufs[oc % 2]
            for m in range(4):
                ps_q = pp.tile([128, OCT * HD], f32, tag="proj", name="ps_q")
                for bc in (0, 4):
                    for k in range(4):
                        nc.tensor.matmul(
                            ps_q[:, bc * HD:(bc + 4) * HD],
                            lhsT=w_sb[:, k, m * 128:(m + 1) * 128],
                            rhs=eT_sb[:, bc:bc + 4, k, 0:NQ],
                            start=(k == 0),
                            stop=(k == 3),
                        )
                nc.scalar.copy(out=qz[0:64, m, 0, :, :], in_=ps_q[0:64, :])
                nc.scalar.copy(out=qz[64:128, m, 1, :, :], in_=ps_q[64:128, :])

            for pr in range(PAIRS_PER_OCT):
                lb = pr * 2          # local batch index in octet
                gpair = oc * PAIRS_PER_OCT + pr

                # ---- K^T ----
                kT_sb = p_kT.tile([128, 4, 2, NE], bf16, tag="kT")
                for m in range(4):
                    ps_k = pp.tile([128, 2 * NE], f32, tag="proj", name="ps_k")
                    for k in range(4):
                        nc.tensor.matmul(
                            ps_k,
                            lhsT=w_sb[:, k, ED + m * 128:ED + (m + 1) * 128],
                            rhs=eT_sb[:, lb:lb + 2, k, :],
                            start=(k == 0),
                            stop=(k == 3),
                        )
                    if m % 2 == 0:
                        nc.vector.tensor_copy(out=kT_sb[:, m, :, :], in_=ps_k)
                    else:
                        nc.scalar.copy(out=kT_sb[:, m, :, :], in_=ps_k)

                # ---- V ----
                v_sb = p_v.tile([128, 2, 2, ED], bf16, tag="v")
                for n2 in range(2):
                    for b2 in range(2):
                        ps_v = pp.tile([128, ED], f32, tag="proj", name="ps_v")
                        for k in range(4):
                            nc.tensor.matmul(
                                ps_v,
                                lhsT=eT_sb[:, lb + b2, k, n2 * 128:(n2 + 1) * 128],
                                rhs=w_sb[:, k, 2 * ED:3 * ED],
                                start=(k == 0),
                                stop=(k == 3),
                            )
                        nc.scalar.copy(out=v_sb[:, n2, b2, :], in_=ps_v)

                # ---- logits^T + exp + keep-mask ----
                # wm[(b2, n2)] : [128(ne-slice), H*NQ] bf16
                wm = {}
                for n2 in range(2):
                    for b2 in range(2):
                        ps_l = pp.tile([128, H * NQ], f32, tag="logit", name="ps_l", bufs=3)
                        for h in range(H):
                            nc.tensor.matmul(
                                ps_l[:, h * NQ:(h + 1) * NQ],
                                lhsT=kT_sb[:, h // 2, b2,
                                           n2 * 128:(n2 + 1) * 128],
                                rhs=qz[:, h // 2, h % 2, lb + b2, :],
                                start=True,
                                stop=True,
                            )
                        wm_t = p_wm.tile([128, H * NQ], bf16, tag="wm", name="wm_t")
                        nc.scalar.activation(
                            out=wm_t, in_=ps_l,
                            func=mybir.ActivationFunctionType.Exp,
                            scale=SCALE,
                        )
                        keep_rep = keep_sb[:, lb + b2, n2, None, :].broadcast_to(
                            [128, H, NQ])
                        nc.vector.tensor_mul(wm_t, wm_t, keep_rep)
                        wm[(b2, n2)] = wm_t

                # ---- softmax denominators (PE broadcast) + attn ----
                recip = {}
                for b2 in range(2):
                    ps_s = pp.tile([128, H * NQ], f32, tag="sums", name="ps_s", bufs=1)
                    for n2 in range(2):
                        nc.tensor.matmul(
                            ps_s,
                            lhsT=ones_sb,
                            rhs=wm[(b2, n2)],
                            start=(n2 == 0),
                            stop=(n2 == 1),
                        )
                    r_sb = p_recip.tile([128, H * NQ], f32, tag="recip", name="r_sb")
                    nc.vector.reciprocal_approx_fast(out=r_sb, in_=ps_s)
                    recip[b2] = r_sb

                ps_a = pp.tile([128, 512], f32, tag="attn", name="ps_a")
                for hp in range(4):
                    for b2 in range(2):
                        for h2 in range(2):
                            h = 2 * hp + h2
                            col = (hp * 2 + b2) * 64
                            for n2 in range(2):
                                nc.tensor.matmul(
                                    ps_a[h2 * 64:(h2 + 1) * 64, col:col + 64],
                                    lhsT=v_sb[:, n2, b2, h * 64:(h + 1) * 64],
                                    rhs=wm[(b2, n2)][:, h * 64:(h + 1) * 64],
                                    start=(n2 == 0),
                                    stop=(n2 == 1),
                                )

                # normalize -> attn_sb (bf16), layout [128(2-head rows), (hp, b2, q)]
                attn_sb = p_attn.tile([128, 512], bf16, tag="attn_sb")
                for b2 in range(2):
                    for h2 in range(2):
                        rows = slice(h2 * 64, (h2 + 1) * 64)
                        o_ap = attn_sb[rows, :].rearrange(
                            "p (hp b q) -> p hp b q", hp=4, b=2)[:, :, b2, :]
                        i_ap = ps_a[rows, :].rearrange(
                            "p (hp b q) -> p hp b q", hp=4, b=2)[:, :, b2, :]
                        r_ap = recip[b2][rows, :].rearrange(
                            "p (hp x) -> p hp x", hp=4)[:, :, h2 * 64:(h2 + 1) * 64]
                        nc.vector.tensor_mul(o_ap, i_ap, r_ap)

                # ---- output projection + bias + post mask ----
                ps_o = pp.tile([128, OUT_DIM], f32, tag="attn", name="ps_o")
                for t in range(4):
                    nc.tensor.matmul(
                        ps_o,
                        lhsT=attn_sb[:, t * 128:(t + 1) * 128],
                        rhs=wo_sb[:, t, :],
                        start=(t == 0),
                        stop=(t == 3),
                    )
                out_sb = p_out.tile([128, OUT_DIM], f32, tag="out_sb")
                nc.vector.tensor_scalar_mul(
                    out_sb, in0=ps_o, scalar1=postm_sb[:, gpair:gpair + 1])
                nc.vector.scalar_tensor_tensor(
                    out_sb, in0=bias_bc,
                    scalar=postm_sb[:, gpair:gpair + 1],
                    in1=out_sb,
                    op0=mybir.AluOpType.mult, op1=mybir.AluOpType.add)
                nc.sync.dma_start(
                    out=out_d[gpair * 2:gpair * 2 + 2].rearrange("b q d -> (b q) d"),
                    in_=out_sb,
                )
    nc.compile()
    return nc


def _prep_inputs(entities, pre_mask, post_mask, W_in, W_out, b_out):
    """Host-side sharding + layout transforms (not part of timed HW work)."""
    # [oct, 128p, b, k, ne] contiguous per partition row
    eT = entities.reshape(BS // OCT, OCT, NE, 4, 128).transpose(
        0, 4, 1, 3, 2).astype(BF16)
    eT = np.ascontiguousarray(eT)
    keep = (1 - pre_mask).astype(np.float32).reshape(
        BS // OCT, OCT, NQ, 2, 128).transpose(0, 4, 1, 3, 2).astype(BF16)
    keep = np.ascontiguousarray(keep)
    kpost = (1 - post_mask).astype(np.float32)  # [BS, NQ]
    w_in = np.ascontiguousarray(W_in).reshape(4, 128, 3 * ED).astype(BF16)
    w_out = np.ascontiguousarray(W_out).reshape(4, 128, OUT_DIM).astype(BF16)
    b_o = b_out.reshape(1, OUT_DIM).astype(np.float32)
    ident = np.eye(128, dtype=np.float32).astype(BF16)

    in_maps = []
    for c in range(NCORES):
        sl = slice(c * BPC, (c + 1) * BPC)
        # postm: [128 rows=(b2*64+q), NPAIRS]
        kp = kpost[sl].reshape(NPAIRS, 128).T.copy()
        in_maps.append({
            "eT": np.ascontiguousarray(eT[c * NOCT:(c + 1) * NOCT]),
            "keep": np.ascontiguousarray(keep[c * NOCT:(c + 1) * NOCT]),
            "postm": np.ascontiguousarray(kp),
            "ident": ident,
            "w_in": w_in,
            "w_out": w_out,
            "b_out": b_o,
        })
    return in_maps


def kernel(entities, pre_mask, post_mask, W_in, W_out, b_out, trace=False):
    global LAST_RESULT
    from concourse.bass_utils import run_bass_kernel_spmd

    if "nc" not in _BUILT:
        _BUILT["nc"] = _build_nc()
    nc = _BUILT["nc"]

    in_maps = _prep_inputs(np.asarray(entities), np.asarray(pre_mask),
                           np.asarray(post_mask), np.asarray(W_in),
                           np.asarray(W_out), np.asarray(b_out))
    res = run_bass_kernel_spmd(nc, in_maps, core_ids=list(range(NCORES)),
                               trace=trace)
    LAST_RESULT = res
    out = np.concatenate([r["out"] for r in res.results], axis=0)
    return out.astype(np.float32)


# revision 21
# speedup vs baseline: 1.0187x; 1.0187x over previous
"""Trainium2 Bass kernel for nn_EntityAttentionLayer (sparse attention).

Strategy (8 cores, data-parallel over bs):
  - Host side: shard bs across 8 cores (64 items each), pre-transpose
    entities to E^T[in_dim, ne] per batch, cast operands to bf16,
    convert masks to multiplicative keep-masks.
  - On chip, per batch b (processed in pairs, Q in octets of 8):
      K^T[ed, ne]  = (Wk^T E^T)        via lhsT=Wk slices, rhs=E^T
      V[ne, ed]    = E V-proj          via lhsT=E^T slices, rhs=Wv
      Q^T[ed, q]   =                   via lhsT=Wq slices, rhs=E^T[:, :64]
      logits^T[ne, q] per head         lhsT=K^T_h, rhs=Q^T_h  (ne on partitions)
      wm = exp(logits * 1/sqrt(hd))    on ACT (scale folded into activation)
      wm *= keep^T                     on DVE (multiplicative mask, h-broadcast)
      sums broadcast [128, h*q]        PE matmul with all-ones lhsT
      attn^T unnorm [2-heads, hp*b*q]  lhsT=V slices, rhs=wm  (col-tiled pairs)
      attn = attn_unnorm * 1/sums      DVE (approx reciprocal + strided muls)
      out[b*q, out] = attn^T.T @ W_out (+bias & post-mask fused on DVE)
      out *= keep_post (per-partition scalar), DMA out.
  All matmuls bf16 operands, fp32 PSUM accumulation.
"""

import numpy as np
import ml_dtypes

BS, NE, NQ, IN_DIM, ED, OUT_DIM, H, HD = 512, 256, 64, 512, 512, 512, 8, 64
NCORES = 8
BPC = BS // NCORES          # 64 batches per core
OCT = 8                     # batches per super-batch (Q^T amortization)
NOCT = BPC // OCT           # 8
PAIRS_PER_OCT = OCT // 2    # 4
NPAIRS = BPC // 2           # 32
SCALE = 1.0 / float(np.sqrt(HD))

BF16 = ml_dtypes.bfloat16

_BUILT = {}
LAST_RESULT = None


def _build_nc():
    import concourse.tile as tile
    from concourse import bacc, mybir
    from contextlib import ExitStack

    f32 = mybir.dt.float32
    bf16 = mybir.dt.bfloat16

    nc = bacc.Bacc("TRN2", target_bir_lowering=False)

    eT_d = nc.dram_tensor("eT", [NOCT, 128, OCT, 4, NE], bf16, kind="ExternalInput")
    keep_d = nc.dram_tensor("keep", [NOCT, 128, OCT, 2, NQ], bf16, kind="ExternalInput")
    postm_d = nc.dram_tensor("postm", [128, NPAIRS], f32, kind="ExternalInput")
    ident_d = nc.dram_tensor("ident", [128, 128], bf16, kind="ExternalInput")
    w_in_d = nc.dram_tensor("w_in", [4, 128, 3 * ED], bf16, kind="ExternalInput")
    w_out_d = nc.dram_tensor("w_out", [4, 128, OUT_DIM], bf16, kind="ExternalInput")
    b_out_d = nc.dram_tensor("b_out", [1, OUT_DIM], f32, kind="ExternalInput")
    out_d = nc.dram_tensor("out", [BPC, NQ, OUT_DIM], f32, kind="ExternalOutput")

    with ExitStack() as ctx:
        tc = ctx.enter_context(tile.TileContext(nc))
        consts = ctx.enter_context(tc.tile_pool(name="consts", bufs=1))
        p_eT = ctx.enter_context(tc.tile_pool(name="p_eT", bufs=2))
        p_keep = ctx.enter_context(tc.tile_pool(name="p_keep", bufs=2))
        p_kT = ctx.enter_context(tc.tile_pool(name="p_kT", bufs=3))
        p_v = ctx.enter_context(tc.tile_pool(name="p_v", bufs=3))
        p_wm = ctx.enter_context(tc.tile_pool(name="p_wm", bufs=6))
        p_recip = ctx.enter_context(tc.tile_pool(name="p_recip", bufs=2))
        p_attn = ctx.enter_context(tc.tile_pool(name="p_attn", bufs=2))
        p_out = ctx.enter_context(tc.tile_pool(name="p_out", bufs=3))
        pp = ctx.enter_context(tc.tile_pool(name="pp", bufs=2, space="PSUM"))

        # Constants
        w_sb = consts.tile([128, 4, 3 * ED], bf16)
        for wc in range(3):
            nc.sync.dma_start(
                out=w_sb[:, :, wc * ED:(wc + 1) * ED],
                in_=w_in_d[:, :, wc * ED:(wc + 1) * ED].rearrange("k p n -> p k n"))
        wo_sb = consts.tile([128, 4, OUT_DIM], bf16)
        nc.sync.dma_start(out=wo_sb, in_=w_out_d[:, :, :].rearrange("k p n -> p k n"))
        bias_bc = consts.tile([128, OUT_DIM], f32)
        nc.sync.dma_start(out=bias_bc, in_=b_out_d[:, :].to_broadcast([128, OUT_DIM]))
        postm_sb = consts.tile([128, NPAIRS], f32)
        nc.sync.dma_start(out=postm_sb, in_=postm_d[:, :])
        ones_sb = consts.tile([128, 128], bf16)
        nc.vector.memset(ones_sb, 1.0)
        ident_sb = consts.tile([128, 128], bf16)
        nc.sync.dma_start(out=ident_sb, in_=ident_d[:, :])
        # Persistent zero-padded Q^T tiles (manual double buffer by octet
        # parity). Layout [128, m, h2, b, q]: head parity h2 selects which
        # 64-row half holds data; the other half stays zero so logits
        # matmuls can use full K=128 operands at base partition 0
        # (operands at base partition 64 fault on HW).
        qz0 = consts.tile([128, 4, 2, OCT, HD], bf16)
        nc.vector.memset(qz0, 0.0)
        qz1 = consts.tile([128, 4, 2, OCT, HD], bf16)
        nc.vector.memset(qz1, 0.0)
        qz_bufs = [qz0, qz1]

        for oc in range(NOCT):
            eT_sb = p_eT.tile([128, OCT, 4, NE], bf16, tag="eT")
            for hc in range(2):
                nc.sync.dma_start(
                    out=eT_sb[:, hc * 4:(hc + 1) * 4, :, :],
                    in_=eT_d[oc, :, hc * 4:(hc + 1) * 4, :, :],
                )
            keep_sb = p_keep.tile([128, OCT, 2, NQ], bf16, tag="keep")
            nc.sync.dma_start(out=keep_sb, in_=keep_d[oc, :, :, :, :])

            # ---- Q^T for the whole octet: amortize W_q weight loads ----
            qz = qz_bufs[oc % 2]
            for m in range(4):
                ps_q = pp.tile([128, OCT * HD], f32, tag="proj", name="ps_q", bufs=3)
                for bc in (0, 4):
                    for k in range(4):
                        nc.tensor.matmul(
                            ps_q[:, bc * HD:(bc + 4) * HD],
                            lhsT=w_sb[:, k, m * 128:(m + 1) * 128],
                            rhs=eT_sb[:, bc:bc + 4, k, 0:NQ],
                            start=(k == 0),
                            stop=(k == 3),
                        )
                nc.scalar.copy(out=qz[0:64, m, 0, :, :], in_=ps_q[0:64, :])
                nc.scalar.copy(out=qz[64:128, m, 1, :, :], in_=ps_q[64:128, :])

            for pr in range(PAIRS_PER_OCT):
                lb = pr * 2          # local batch index in octet
                gpair = oc * PAIRS_PER_OCT + pr

                # ---- K^T ----
                kT_sb = p_kT.tile([128, 4, 2, NE], bf16, tag="kT")
                for m in range(4):
                    ps_k = pp.tile([128, 2 * NE], f32, tag="proj", name="ps_k", bufs=3)
                    for k in range(4):
                        nc.tensor.matmul(
                            ps_k,
                            lhsT=w_sb[:, k, ED + m * 128:ED + (m + 1) * 128],
                            rhs=eT_sb[:, lb:lb + 2, k, :],
                            start=(k == 0),
                            stop=(k == 3),
                        )
                    if m % 2 == 0:
                        nc.vector.tensor_copy(out=kT_sb[:, m, :, :], in_=ps_k)
                    else:
                        nc.scalar.copy(out=kT_sb[:, m, :, :], in_=ps_k)

                # ---- V ----
                v_sb = p_v.tile([128, 2, 2, ED], bf16, tag="v")
                for n2 in range(2):
                    for b2 in range(2):
                        ps_v = pp.tile([128, ED], f32, tag="proj", name="ps_v", bufs=3)
                        for k in range(4):
                            nc.tensor.matmul(
                                ps_v,
                                lhsT=eT_sb[:, lb + b2, k, n2 * 128:(n2 + 1) * 128],
                                rhs=w_sb[:, k, 2 * ED:3 * ED],
                                start=(k == 0),
                                stop=(k == 3),
                            )
                        nc.scalar.copy(out=v_sb[:, n2, b2, :], in_=ps_v)

                # ---- logits^T + exp + keep-mask ----
                # wm[(b2, n2)] : [128(ne-slice), H*NQ] bf16
                wm = {}
                for n2 in range(2):
                    for b2 in range(2):
                        ps_l = pp.tile([128, H * NQ], f32, tag="logit", name="ps_l", bufs=2)
                        for h in range(H):
                            nc.tensor.matmul(
                                ps_l[:, h * NQ:(h + 1) * NQ],
                                lhsT=kT_sb[:, h // 2, b2,
                                           n2 * 128:(n2 + 1) * 128],
                                rhs=qz[:, h // 2, h % 2, lb + b2, :],
                                start=True,
                                stop=True,
                            )
                        wm_t = p_wm.tile([128, H * NQ], bf16, tag="wm", name="wm_t")
                        nc.scalar.activation(
                            out=wm_t, in_=ps_l,
                            func=mybir.ActivationFunctionType.Exp,
                            scale=SCALE,
                        )
                        keep_rep = keep_sb[:, lb + b2, n2, None, :].broadcast_to(
                            [128, H, NQ])
                        nc.vector.tensor_mul(wm_t, wm_t, keep_rep)
                        wm[(b2, n2)] = wm_t

                # ---- softmax denominators (PE broadcast) + attn ----
                recip = {}
                for b2 in range(2):
                    ps_s = pp.tile([128, H * NQ], f32, tag="sums", name="ps_s", bufs=1)
                    for n2 in range(2):
                        nc.tensor.matmul(
                            ps_s,
                            lhsT=ones_sb,
                            rhs=wm[(b2, n2)],
                            start=(n2 == 0),
                            stop=(n2 == 1),
                        )
                    r_sb = p_recip.tile([128, H * NQ], f32, tag="recip", name="r_sb")
                    nc.vector.reciprocal_approx_fast(out=r_sb, in_=ps_s)
                    recip[b2] = r_sb

                ps_a = pp.tile([128, 512], f32, tag="attn", name="ps_a")
                for hp in range(4):
                    for b2 in range(2):
                        for h2 in range(2):
                            h = 2 * hp + h2
                            col = (hp * 2 + b2) * 64
                            for n2 in range(2):
                                nc.tensor.matmul(
                                    ps_a[h2 * 64:(h2 + 1) * 64, col:col + 64],
                                    lhsT=v_sb[:, n2, b2, h * 64:(h + 1) * 64],
                                    rhs=wm[(b2, n2)][:, h * 64:(h + 1) * 64],
                                    start=(n2 == 0),
                                    stop=(n2 == 1),
                                )

                # normalize -> attn_sb (bf16), layout [128(2-head rows), (hp, b2, q)]
                attn_sb = p_attn.tile([128, 512], bf16, tag="attn_sb")
                for b2 in range(2):
                    for h2 in range(2):
                        rows = slice(h2 * 64, (h2 + 1) * 64)
                        o_ap = attn_sb[rows, :].rearrange(
                            "p (hp b q) -> p hp b q", hp=4, b=2)[:, :, b2, :]
                        i_ap = ps_a[rows, :].rearrange(
                            "p (hp b q) -> p hp b q", hp=4, b=2)[:, :, b2, :]
                        r_ap = recip[b2][rows, :].rearrange(
                            "p (hp x) -> p hp x", hp=4)[:, :, h2 * 64:(h2 + 1) * 64]
                        nc.vector.tensor_mul(o_ap, i_ap, r_ap)

                # ---- output projection + bias + post mask ----
                ps_o = pp.tile([128, OUT_DIM], f32, tag="attn", name="ps_o")
                for t in range(4):
                    nc.tensor.matmul(
                        ps_o,
                        lhsT=attn_sb[:, t * 128:(t + 1) * 128],
                        rhs=wo_sb[:, t, :],
                        start=(t == 0),
                        stop=(t == 3),
                    )
                out_sb = p_out.tile([128, OUT_DIM], f32, tag="out_sb")
                nc.vector.tensor_scalar_mul(
                    out_sb, in0=ps_o, scalar1=postm_sb[:, gpair:gpair + 1])
                nc.vector.scalar_tensor_tensor(
                    out_sb, in0=bias_bc,
                    scalar=postm_sb[:, gpair:gpair + 1],
                    in1=out_sb,
                    op0=mybir.AluOpType.mult, op1=mybir.AluOpType.add)
                nc.sync.dma_start(
                    out=out_d[gpair * 2:gpair * 2 + 2].rearrange("b q d -> (b q) d"),
                    in_=out_sb,
                )
    nc.compile()
    return nc


def _prep_inputs(entities, pre_mask, post_mask, W_in, W_out, b_out):
    """Host-side sharding + layout transforms (not part of timed HW work)."""
    # [oct, 128p, b, k, ne] contiguous per partition row
    eT = entities.reshape(BS // OCT, OCT, NE, 4, 128).transpose(
        0, 4, 1, 3, 2).astype(BF16)
    eT = np.ascontiguousarray(eT)
    keep = (1 - pre_mask).astype(np.float32).reshape(
        BS // OCT, OCT, NQ, 2, 128).transpose(0, 4, 1, 3, 2).astype(BF16)
    keep = np.ascontiguousarray(keep)
    kpost = (1 - post_mask).astype(np.float32)  # [BS, NQ]
    w_in = np.ascontiguousarray(W_in).reshape(4, 128, 3 * ED).astype(BF16)
    w_out = np.ascontiguousarray(W_out).reshape(4, 128, OUT_DIM).astype(BF16)
    b_o = b_out.reshape(1, OUT_DIM).astype(np.float32)
    ident = np.eye(128, dtype=np.float32).astype(BF16)

    in_maps = []
    for c in range(NCORES):
        sl = slice(c * BPC, (c + 1) * BPC)
        # postm: [128 rows=(b2*64+q), NPAIRS]
        kp = kpost[sl].reshape(NPAIRS, 128).T.copy()
        in_maps.append({
            "eT": np.ascontiguousarray(eT[c * NOCT:(c + 1) * NOCT]),
            "keep": np.ascontiguousarray(keep[c * NOCT:(c + 1) * NOCT]),
            "postm": np.ascontiguousarray(kp),
            "ident": ident,
            "w_in": w_in,
            "w_out": w_out,
            "b_out": b_o,
        })
    return in_maps


def kernel(entities, pre_mask, post_mask, W_in, W_out, b_out, trace=False):
    global LAST_RESULT
    from concourse.bass_utils import run_bass_kernel_spmd

    if "nc" not in _BUILT:
        _BUILT["nc"] = _build_nc()
    nc = _BUILT["nc"]

    in_maps = _prep_inputs(np.asarray(entities), np.asarray(pre_mask),
                           np.asarray(post_mask), np.asarray(W_in),
                           np.asarray(W_out), np.asarray(b_out))
    res = run_bass_kernel_spmd(nc, in_maps, core_ids=list(range(NCORES)),
                               trace=trace)
    LAST_RESULT = res
    out = np.concatenate([r["out"] for r in res.results], axis=0)
    return out.astype(np.float32)
